# revision 1
# baseline (speedup 1.0000x reference)
"""Per-pixel predicted 5x5 conv (KPN-style) on 8 trn2 cores.

Sharding: data-parallel over (batch x H-half) = 8 shards, halo rows included
in each shard's input slice (host-side zero-padded, so no edge cases).

Device layout (per core):
  partitions = 128 output rows (h), free = (c, w) c-major.
  - 10 SBUF copies of the feat slice: 5 h-shifts (di) x 2 w-parities, so every
    tap (di, dj) is a clean slice with 4B-aligned, stride-1 inner w runs ->
    DVE tensor_tensor runs in 2x_1P bf16 mode.
  - per tap: DVE bf16 multiply prod = feat_shift * kernel_tap (kernel tap
    broadcast across c via stride-0 AP dim).
  - 25-tap accumulation: PE identity-matmul PSUM accumulate (start on a bias
    matmul, so bias rides along for free).
  - ACT evacuates PSUM -> SBUF fp32, DMA out.
"""

import sys

for p in ("/opt/pypackages", "/opt/trn_rl_repo"):
    if p not in sys.path:
        sys.path.insert(0, p)

import numpy as np
import ml_dtypes

import concourse.mybir as mybir
from concourse import bacc, tile
from concourse.bass_utils import run_bass_kernel_spmd

B, H, W, C, KK, K = 4, 256, 256, 32, 25, 5
HS = H // 2          # 128 output rows per core
WPAD = W + 8         # w index j == original w (j-2); zeros outside
CH = 16              # channels per half-pass (SBUF fit)
CQ = 8               # channels per PSUM chunk (4 banks)
BF16 = mybir.dt.bfloat16
F32 = mybir.dt.float32

_NC_CACHE = {}


def _build_nc():
    nc = bacc.Bacc(None, target_bir_lowering=False)
    feat_d = nc.dram_tensor("feat", [HS + 4, C, WPAD], BF16, kind="ExternalInput")
    kern_d = nc.dram_tensor("kern", [HS, KK, W], BF16, kind="ExternalInput")
    bias_d = nc.dram_tensor("biasr", [128, C, W], BF16, kind="ExternalInput")
    iden_d = nc.dram_tensor("iden", [128, 128], BF16, kind="ExternalInput")
    out_d = nc.dram_tensor("out", [HS, C, W], F32, kind="ExternalOutput")

    with tile.TileContext(nc) as tc:
        with tc.tile_pool(name="const", bufs=1) as cpool, \
             tc.tile_pool(name="copies", bufs=2) as fpool, \
             tc.tile_pool(name="prod", bufs=6) as ppool, \
             tc.tile_pool(name="osb", bufs=4) as opool, \
             tc.tile_pool(name="psum", bufs=2, space="PSUM") as qpool:
            ident = cpool.tile([128, 128], BF16, tag="ident")
            nc.sync.dma_start(out=ident, in_=iden_d[:, :])
            kern_t = cpool.tile([128, KK, W], BF16, tag="kern")
            nc.sync.dma_start(out=kern_t, in_=kern_d[:, :, :])
            bias_t = cpool.tile([128, C, W], BF16, tag="bias")
            nc.sync.dma_start(out=bias_t, in_=bias_d[:, :, :])

            for qp in range(C // CQ):          # quarter-pass = one PSUM chunk
                cq0 = qp * CQ
                cops = {}
                for di in range(K):
                    for par in range(2):
                        t = fpool.tile([128, CQ, W + 4], BF16,
                                       tag=f"cop{di}_{par}")
                        nc.sync.dma_start(
                            out=t,
                            in_=feat_d[di:di + 128, cq0:cq0 + CQ,
                                       par:par + W + 4])
                        cops[(di, par)] = t
                psum_t = qpool.tile([128, 4, 512], F32, tag="ps")
                # bias seeds the accumulation group (start=True)
                for j in range(4):
                    nc.tensor.matmul(
                        psum_t[:, j:j + 1, :],
                        ident,
                        bias_t[:, cq0 + 2 * j:cq0 + 2 * j + 2, :],
                        start=True, stop=False)
                for ti in range(KK):
                    di, dj = ti // K, ti % K
                    par = dj % 2
                    s = dj - par
                    cop = cops[(di, par)]
                    prod = ppool.tile([128, CQ, W], BF16, tag="prod")
                    in0 = cop[:, :, s:s + W]
                    in1 = kern_t[:, ti:ti + 1, :].broadcast_to(
                        (128, CQ, W))
                    nc.vector.tensor_tensor(prod, in0, in1,
                                            mybir.AluOpType.mult)
                    last = ti == KK - 1
                    for j in range(4):
                        nc.tensor.matmul(
                            psum_t[:, j:j + 1, :],
                            ident,
                            prod[:, 2 * j:2 * j + 2, :],
                            start=False, stop=last)
                for j in range(4):
                    out_sb = opool.tile([128, 2, W], F32, tag="osb")
                    nc.scalar.copy(
                        out=out_sb.rearrange("p a b -> p (a b)"),
                        in_=psum_t[:, j:j + 1, :].rearrange(
                            "p a b -> p (a b)"))
                    nc.sync.dma_start(
                        out=out_d[:, cq0 + 2 * j:cq0 + 2 * j + 2, :],
                        in_=out_sb)
    if not nc.is_finalized():
        nc.finalize()
    return nc


def _get_nc():
    if "nc" not in _NC_CACHE:
        _NC_CACHE["nc"] = _build_nc()
    return _NC_CACHE["nc"]


def _prep_inputs(feat, kernel, bias):
    ft = np.ascontiguousarray(feat.transpose(0, 1, 3, 2))   # [B, H, C, W]
    fp = np.zeros((B, H + 4, C, WPAD), np.float32)
    fp[:, 2:H + 2, :, 2:W + 2] = ft
    fpb = fp.astype(ml_dtypes.bfloat16)
    kt = np.ascontiguousarray(
        kernel.transpose(0, 1, 3, 2)).astype(ml_dtypes.bfloat16)  # [B,H,25,W]
    biasr = np.ascontiguousarray(
        np.broadcast_to(
            bias.astype(ml_dtypes.bfloat16)[None, :, None], (128, C, W)))
    iden = np.eye(128, dtype=ml_dtypes.bfloat16)
    in_maps = []
    for core in range(8):
        b, hh = core // 2, core % 2
        h0 = hh * HS
        in_maps.append({
            "feat": np.ascontiguousarray(fpb[b, h0:h0 + HS + 4]),
            "kern": np.ascontiguousarray(kt[b, h0:h0 + HS]),
            "biasr": biasr,
            "iden": iden,
        })
    return in_maps


def _run(feat, kernel, bias, **run_kwargs):
    nc = _get_nc()
    in_maps = _prep_inputs(feat, kernel, bias)
    res = run_bass_kernel_spmd(nc, in_maps, core_ids=list(range(8)),
                               **run_kwargs)
    out = np.empty((B, H, C, W), np.float32)
    for core in range(8):
        b, hh = core // 2, core % 2
        out[b, hh * HS:(hh + 1) * HS] = res.results[core]["out"]
    return np.ascontiguousarray(out.transpose(0, 1, 3, 2)), res


def kernel(feat, kernel, bias):
    out, _ = _run(np.asarray(feat, np.float32), np.asarray(kernel, np.float32),
                  np.asarray(bias, np.float32))
    return out



# revision 5
# speedup vs baseline: 2.2991x; 2.2991x over previous
"""Per-pixel predicted 5x5 conv (KPN) on 8 trn2 cores.

Deep-contraction im2col: each 8x4 output tile (one window) is ONE PE matmul
contracting over the 12x8 input patch (96 partitions):

  out[c, t] = sum_p  S[p, c] * M[p, t]
  S[p=(dh,du), c]       = feat[8wh+dh-2, 4ww+du-2, c]   (host-gathered slabs)
  M[p=(dh,du), t=(th,tw)] = kernel[8wh+th, 4ww+tw, (dh-th)*5+(du-tw)]
                            if both tap offsets in [0,5) else 0  (banded)

All 25 taps of 32 output pixels x 32 channels finish in one 32-column
matmul (bf16 in, fp32 PSUM). ACT evacuates PSUM with the bias add fused;
output leaves as bf16. DMA per core: S 6.3MB + M 6.3MB + out 2.1MB.
"""

import sys

for p in ("/opt/pypackages", "/opt/trn_rl_repo"):
    if p not in sys.path:
        sys.path.insert(0, p)

import numpy as np
import ml_dtypes

import concourse.mybir as mybir
from concourse import bacc, tile
from concourse.bass_utils import run_bass_kernel_spmd

B, H, W, C, KK, K = 4, 256, 256, 32, 25, 5
HS = H // 2            # 128 output rows per core
TH, TW = 8, 4          # output tile per window
PH, PU = TH + 4, TW + 4    # input patch dims -> 12*8 = 96 partitions
NP = PH * PU           # 96
NWH, NWW = HS // TH, W // TW   # 16 x 64 = 1024 windows per core
NT = TH * TW           # 32 moving columns per window
SLOTS = 15             # windows of one rho-quadrant per PSUM bank (15*32=480)
NG = (NWH * NWW // 4 + SLOTS - 1) // SLOTS   # 18 evac groups (last partial)
OUTF = (NWH * NWW // 4) * NT  # 8192 free elems per out partition
BF16 = mybir.dt.bfloat16
F32 = mybir.dt.float32

_NC_CACHE = {}


def _build_nc():
    nc = bacc.Bacc(None, target_bir_lowering=False)
    s_d = nc.dram_tensor("s", [NWH, NP, NWW * C], BF16, kind="ExternalInput")
    m_d = nc.dram_tensor("m", [NWH, NP, NWW * NT], BF16, kind="ExternalInput")
    bias_d = nc.dram_tensor("biasr", [128, 1], F32, kind="ExternalInput")
    out_d = nc.dram_tensor("out", [128, OUTF], BF16, kind="ExternalOutput")

    with tile.TileContext(nc) as tc:
        with tc.tile_pool(name="const", bufs=1) as cpool, \
             tc.tile_pool(name="sm", bufs=3) as spool, \
             tc.tile_pool(name="osb", bufs=4) as opool, \
             tc.tile_pool(name="psum", bufs=4, space="PSUM") as qpool:
            bias_t = cpool.tile([128, 1], F32, tag="bias")
            nc.sync.dma_start(out=bias_t, in_=bias_d[:, :])

            ps = None
            for wh in range(NWH):
                s_t = spool.tile([NP, NWW, C], BF16, tag="s")
                nc.sync.dma_start(out=s_t, in_=s_d[wh, :, :])
                m_t = spool.tile([NP, NWW, NT], BF16, tag="m")
                nc.sync.dma_start(out=m_t, in_=m_d[wh, :, :])
                for ww in range(NWW):
                    w = wh * NWW + ww        # window index
                    rho, sg = w % 4, w // 4
                    g, slot = sg // SLOTS, sg % SLOTS
                    if rho == 0 and slot == 0:
                        ps = qpool.tile([128, SLOTS * NT], F32, tag="ps")
                    nc.tensor.matmul(
                        ps[32 * rho:32 * rho + 32,
                           slot * NT:(slot + 1) * NT],
                        s_t[:, ww, :],
                        m_t[:, ww, :],
                        start=True, stop=True,
                        tile_position=(0, 32 * rho))
                    if w == NWH * NWW - 1 or (rho == 3 and slot == SLOTS - 1):
                        nf = (slot + 1) * NT
                        ob = opool.tile([128, SLOTS * NT], BF16, tag="ob")
                        nc.scalar.activation(
                            ob[:, :nf], ps[:, :nf],
                            mybir.ActivationFunctionType.Identity,
                            bias=bias_t[:, :], scale=1.0)
                        nc.sync.dma_start(
                            out=out_d[:, g * SLOTS * NT:g * SLOTS * NT + nf],
                            in_=ob[:, :nf])
    if not nc.is_finalized():
        nc.finalize()
    return nc


def _get_nc():
    if "nc" not in _NC_CACHE:
        _NC_CACHE["nc"] = _build_nc()
    return _NC_CACHE["nc"]


def _prep_inputs(feat, kernel, bias):
    fb = feat.astype(ml_dtypes.bfloat16)
    kb = kernel.astype(ml_dtypes.bfloat16)
    # broadcastable index arrays over [wh, dh, du, ww, th, tw]
    IH = (8 * np.arange(NWH)[:, None, None, None, None, None]
          + np.arange(TH)[None, None, None, None, :, None]).astype(np.int32)
    IW = (4 * np.arange(NWW)[None, None, None, :, None, None]
          + np.arange(TW)[None, None, None, None, None, :]).astype(np.int32)
    IDI = (np.arange(PH)[None, :, None, None, None, None]
           - np.arange(TH)[None, None, None, None, :, None] + 7).astype(np.int32)
    IDJ = (np.arange(PU)[None, None, :, None, None, None]
           - np.arange(TW)[None, None, None, None, None, :] + 3).astype(np.int32)

    bias_rep = np.ascontiguousarray(
        np.tile(bias.astype(np.float32), 4)[:, None])    # [128,1]

    in_maps = []
    for core in range(8):
        b, hh = core // 2, core % 2
        # padded feat rows [-2, 130) x cols [-2, 258)
        fpad = np.zeros((PH + 8 * (NWH - 1), W + 4, C), ml_dtypes.bfloat16)
        r0 = hh * HS - 2
        lo, hi = max(0, -r0), min(132, H - r0)
        fpad[lo:hi, 2:W + 2] = fb[b, r0 + lo:r0 + hi]
        # S[wh, p=(dh,du), ww, c] = fpad[8wh+dh, 4ww+du, c]
        s_arr = fpad[(8 * np.arange(NWH)[:, None, None, None]
                      + np.arange(PH)[None, :, None, None]),
                     (4 * np.arange(NWW)[None, None, None, :]
                      + np.arange(PU)[None, None, :, None])]  # [wh,dh,du,ww,c]
        s_arr = s_arr.reshape(NWH, NP, NWW * C)
        # padded tap table for this core
        kp2 = np.zeros((HS, W, PH + TH - 1, PU + TW - 1), ml_dtypes.bfloat16)
        kc = kb[b, hh * HS:(hh + 1) * HS]                # [128, 256, 25]
        for di in range(K):
            for dj in range(K):
                kp2[:, :, di + 7, dj + 3] = kc[:, :, di * K + dj]
        m_arr = kp2[IH, IW, IDI, IDJ]                    # [wh,dh,du,ww,th,tw]
        m_arr = m_arr.reshape(NWH, NP, NWW * NT)
        in_maps.append({
            "s": np.ascontiguousarray(s_arr),
            "m": np.ascontiguousarray(m_arr),
            "biasr": bias_rep,
        })
    return in_maps


def _unshard(results):
    out = np.empty((B, H, W, C), np.float32)
    for core in range(8):
        b, hh = core // 2, core % 2
        res = np.asarray(results[core]["out"], ml_dtypes.bfloat16)
        r4 = res.reshape(4, C, NWH * NWW // 4, TH, TW)   # [rho,c,sg,th,tw]
        oc = np.empty((NWH, TH, NWW, TW, C), np.float32)
        for rho in range(4):
            # sg = 16*wh + s'  ->  ww = 4*s' + rho
            blk = r4[rho].reshape(C, NWH, NWW // 4, TH, TW)
            oc[:, :, rho::4, :, :] = blk.transpose(1, 3, 2, 4, 0)
        out[b, hh * HS:(hh + 1) * HS] = oc.reshape(HS, W, C)
    return out


def _run(feat, kernel, bias, **run_kwargs):
    nc = _get_nc()
    in_maps = _prep_inputs(feat, kernel, bias)
    res = run_bass_kernel_spmd(nc, in_maps, core_ids=list(range(8)),
                               **run_kwargs)
    return _unshard(res.results), res


def kernel(feat, kernel, bias):
    out, _ = _run(np.asarray(feat, np.float32), np.asarray(kernel, np.float32),
                  np.asarray(bias, np.float32))
    return out


# revision 9
# speedup vs baseline: 2.8588x; 1.2434x over previous
"""Per-pixel predicted 5x5 conv (KPN) on 8 trn2 cores.

Deep-contraction im2col: each 8x4 output tile (one window) is ONE PE matmul
contracting over the 12x8 input patch (96 partitions):

  out[c, t] = sum_p  S[p, c] * M[p, t]
  S[p=(dh,du), c]       = feat[8wh+dh-2, 4ww+du-2, c]   (host-gathered slabs)
  M[p=(dh,du), t=(th,tw)] = kernel[8wh+th, 4ww+tw, (dh-th)*5+(du-tw)]
                            if both tap offsets in [0,5) else 0  (banded)

All 25 taps of 32 output pixels x 32 channels finish in one 32-column
matmul (bf16 in, fp32 PSUM). ACT evacuates PSUM with the bias add fused;
output leaves as bf16. DMA per core: S 6.3MB + M 6.3MB + out 2.1MB.
"""

import sys

for p in ("/opt/pypackages", "/opt/trn_rl_repo"):
    if p not in sys.path:
        sys.path.insert(0, p)

import numpy as np
import ml_dtypes

import concourse.mybir as mybir
from concourse import bacc, tile
from concourse.bass_utils import run_bass_kernel_spmd

B, H, W, C, KK, K = 4, 256, 256, 32, 25, 5
HS = H // 2            # 128 output rows per core
TH, TW = 8, 4          # output tile per window
PH, PU = TH + 4, TW + 4    # input patch dims -> 12*8 = 96 partitions
NP = PH * PU           # 96
NWH, NWW = HS // TH, W // TW   # 16 x 64 = 1024 windows per core
NT = TH * TW           # 32 moving columns per window
SLOTS = 15             # windows of one rho-quadrant per PSUM bank (15*32=480)
NG = (NWH * NWW // 4 + SLOTS - 1) // SLOTS   # 18 evac groups (last partial)
OUTF = (NWH * NWW // 4) * NT  # 8192 free elems per out partition
BF16 = mybir.dt.bfloat16
F32 = mybir.dt.float32

_NC_CACHE = {}


def _build_nc():
    nc = bacc.Bacc(None, target_bir_lowering=False)
    s_d = nc.dram_tensor("s", [NWH // 2, NP, 2 * NWW * C], BF16,
                         kind="ExternalInput")
    m_d = nc.dram_tensor("m", [NWH // 2, NP, 2 * NWW * NT], BF16,
                         kind="ExternalInput")
    bias_d = nc.dram_tensor("biasr", [128, 1], F32, kind="ExternalInput")
    out_d = nc.dram_tensor("out", [128, OUTF], BF16, kind="ExternalOutput")
    GNT = SLOTS * NT                      # 480 out elems per evac group

    with tile.TileContext(nc) as tc:
        with tc.tile_pool(name="const", bufs=1) as cpool, \
             tc.tile_pool(name="sm", bufs=3) as spool, \
             tc.tile_pool(name="osb", bufs=4) as opool, \
             tc.tile_pool(name="psum", bufs=4, space="PSUM") as qpool:
            bias_t = cpool.tile([128, 1], F32, tag="bias")
            nc.sync.dma_start(out=bias_t, in_=bias_d[:, :])

            ps = ob = None
            for wh in range(NWH):
                if wh % 2 == 0:
                    s_t = spool.tile([NP, 2, NWW, C], BF16, tag="s")
                    nc.sync.dma_start(out=s_t, in_=s_d[wh // 2, :, :])
                    m_t = spool.tile([NP, 2, NWW, NT], BF16, tag="m")
                    nc.sync.dma_start(out=m_t, in_=m_d[wh // 2, :, :])
                for ww in range(NWW):
                    w = wh * NWW + ww        # window index
                    rho, sg = w % 4, w // 4
                    g, slot = sg // SLOTS, sg % SLOTS
                    if rho == 0 and slot == 0:
                        ps = qpool.tile([128, GNT], F32, tag="ps")
                    nc.tensor.matmul(
                        ps[32 * rho:32 * rho + 32,
                           slot * NT:(slot + 1) * NT],
                        s_t[:, wh % 2, ww, :],
                        m_t[:, wh % 2, ww, :],
                        start=True, stop=True,
                        tile_position=(0, 32 * rho))
                    if w == NWH * NWW - 1 or (rho == 3 and slot == SLOTS - 1):
                        nf = (slot + 1) * NT
                        if g % 2 == 0:
                            ob = opool.tile([128, 2 * GNT], BF16, tag="ob")
                        nc.scalar.activation(
                            ob[:, (g % 2) * GNT:(g % 2) * GNT + nf],
                            ps[:, :nf],
                            mybir.ActivationFunctionType.Identity,
                            bias=bias_t[:, :], scale=1.0)
                        if g % 2 == 1 or w == NWH * NWW - 1:
                            g0 = g - (g % 2)
                            tot = (g % 2) * GNT + nf
                            nc.gpsimd.dma_start(
                                out=out_d[:, g0 * GNT:g0 * GNT + tot],
                                in_=ob[:, :tot])
    if not nc.is_finalized():
        nc.finalize()
    return nc


def _get_nc():
    if "nc" not in _NC_CACHE:
        _NC_CACHE["nc"] = _build_nc()
    return _NC_CACHE["nc"]


def _prep_inputs(feat, kernel, bias):
    fb = feat.astype(ml_dtypes.bfloat16)
    kb = kernel.astype(ml_dtypes.bfloat16)
    # broadcastable index arrays over [wh, dh, du, ww, th, tw]
    IH = (8 * np.arange(NWH)[:, None, None, None, None, None]
          + np.arange(TH)[None, None, None, None, :, None]).astype(np.int32)
    IW = (4 * np.arange(NWW)[None, None, None, :, None, None]
          + np.arange(TW)[None, None, None, None, None, :]).astype(np.int32)
    IDI = (np.arange(PH)[None, :, None, None, None, None]
           - np.arange(TH)[None, None, None, None, :, None] + 7).astype(np.int32)
    IDJ = (np.arange(PU)[None, None, :, None, None, None]
           - np.arange(TW)[None, None, None, None, None, :] + 3).astype(np.int32)

    bias_rep = np.ascontiguousarray(
        np.tile(bias.astype(np.float32), 4)[:, None])    # [128,1]

    in_maps = []
    for core in range(8):
        b, hh = core // 2, core % 2
        # padded feat rows [-2, 130) x cols [-2, 258)
        fpad = np.zeros((PH + 8 * (NWH - 1), W + 4, C), ml_dtypes.bfloat16)
        r0 = hh * HS - 2
        lo, hi = max(0, -r0), min(132, H - r0)
        fpad[lo:hi, 2:W + 2] = fb[b, r0 + lo:r0 + hi]
        # S[wh, p=(dh,du), ww, c] = fpad[8wh+dh, 4ww+du, c]
        s_arr = fpad[(8 * np.arange(NWH)[:, None, None, None]
                      + np.arange(PH)[None, :, None, None]),
                     (4 * np.arange(NWW)[None, None, None, :]
                      + np.arange(PU)[None, None, :, None])]  # [wh,dh,du,ww,c]
        s_arr = (s_arr.reshape(NWH // 2, 2, NP, NWW * C)
                 .transpose(0, 2, 1, 3).reshape(NWH // 2, NP, 2 * NWW * C))
        # padded tap table for this core
        kp2 = np.zeros((HS, W, PH + TH - 1, PU + TW - 1), ml_dtypes.bfloat16)
        kc = kb[b, hh * HS:(hh + 1) * HS]                # [128, 256, 25]
        for di in range(K):
            for dj in range(K):
                kp2[:, :, di + 7, dj + 3] = kc[:, :, di * K + dj]
        m_arr = kp2[IH, IW, IDI, IDJ]                    # [wh,dh,du,ww,th,tw]
        m_arr = (m_arr.reshape(NWH // 2, 2, NP, NWW * NT)
                 .transpose(0, 2, 1, 3).reshape(NWH // 2, NP, 2 * NWW * NT))
        in_maps.append({
            "s": np.ascontiguousarray(s_arr),
            "m": np.ascontiguousarray(m_arr),
            "biasr": bias_rep,
        })
    return in_maps


def _unshard(results):
    out = np.empty((B, H, W, C), np.float32)
    for core in range(8):
        b, hh = core // 2, core % 2
        res = np.asarray(results[core]["out"], ml_dtypes.bfloat16)
        r4 = res.reshape(4, C, NWH * NWW // 4, TH, TW)   # [rho,c,sg,th,tw]
        oc = np.empty((NWH, TH, NWW, TW, C), np.float32)
        for rho in range(4):
            # sg = 16*wh + s'  ->  ww = 4*s' + rho
            blk = r4[rho].reshape(C, NWH, NWW // 4, TH, TW)
            oc[:, :, rho::4, :, :] = blk.transpose(1, 3, 2, 4, 0)
        out[b, hh * HS:(hh + 1) * HS] = oc.reshape(HS, W, C)
    return out


def _run(feat, kernel, bias, **run_kwargs):
    nc = _get_nc()
    in_maps = _prep_inputs(feat, kernel, bias)
    res = run_bass_kernel_spmd(nc, in_maps, core_ids=list(range(8)),
                               **run_kwargs)
    return _unshard(res.results), res


def kernel(feat, kernel, bias):
    out, _ = _run(np.asarray(feat, np.float32), np.asarray(kernel, np.float32),
                  np.asarray(bias, np.float32))
    return out
